# revision 1
# baseline (speedup 1.0000x reference)
"""BinsChamferLoss Trainium2 kernel.

Problem: bins [4,257], target_depth_maps [4,240,320] ->
scalar chamfer loss between per-image bin centers (256 1-D points) and
the valid depth pixels (76800 1-D points per image).

Sharding: the 76800-pixel dim is split across 8 cores (9600 pixels each).
Each core computes, for all 4 images:
  - acc_x [128,256]: per-(partition,bin) running min of (bc - t)^2 over its
    local points (partition p owns batch p//32, 300 points each)
  - dy sums + valid counts per partition (cham_y partials)
Host combines: min over cores/partitions -> cham_x, sums -> cham_y.

Invalid points (t < 0.001) are pushed 1e9 away so they never win a min and
their dy is masked out of the sum.
"""

import os
import sys

import numpy as np

sys.path.insert(0, "/opt/trn_rl_repo")

N_CORES = 8
N, P = 4, 256  # batches, bins
L = 240 * 320  # 76800 points per batch
L_LOC = L // N_CORES  # 9600 per core
COLS = (N * L_LOC) // 128  # 300 point-columns per partition
PARTS_PER_BATCH = 128 // N  # 32

_CACHE = {}


def _build_program():
    import concourse.bacc as bacc
    import concourse.tile as tile
    from concourse import mybir

    f32 = mybir.dt.float32
    Alu = mybir.AluOpType
    Act = mybir.ActivationFunctionType

    nc = bacc.Bacc("TRN2", target_bir_lowering=False, debug=False,
                   num_devices=N_CORES)
    tpd = nc.dram_tensor("tpd", [N * L_LOC], f32, kind="ExternalInput").ap()
    bct = nc.dram_tensor("bct", [128, P], f32, kind="ExternalInput").ap()
    outx = nc.dram_tensor("outx", [128, P], f32, kind="ExternalOutput").ap()
    outy = nc.dram_tensor("outy", [128, 2], f32, kind="ExternalOutput").ap()

    with tile.TileContext(nc) as tc:
        with tc.tile_pool(name="consts", bufs=1) as consts, \
             tc.tile_pool(name="work", bufs=6) as work:
            bct_sb = consts.tile([128, P], f32, tag="bct")
            nc.sync.dma_start(bct_sb[:], bct)
            tp_sb = consts.tile([128, COLS], f32, tag="tp")
            nc.sync.dma_start(tp_sb[:], tpd.rearrange("(p c) -> p c", p=128))

            # valid = (t >= 0.001) as 1.0/0.0
            valid = consts.tile([128, COLS], f32, tag="valid")
            nc.vector.tensor_scalar(valid[:], tp_sb[:], 0.001, None,
                                    op0=Alu.is_ge)
            # tneg = -(t_adj); t_adj = t (valid) / t + 1e9 (invalid)
            # tmp = valid*1e9 - 1e9  (0 for valid, -1e9 for invalid)
            tmp = work.tile([128, COLS], f32, tag="tmp")
            nc.vector.tensor_scalar(tmp[:], valid[:], 1e9, -1e9,
                                    op0=Alu.mult, op1=Alu.add)
            tneg = consts.tile([128, COLS], f32, tag="tneg")
            nc.vector.tensor_sub(tneg[:], tmp[:], tp_sb[:])

            acc_x = consts.tile([128, P], f32, tag="accx")
            nc.vector.memset(acc_x[:], 3.0e38)
            dy = consts.tile([128, COLS], f32, tag="dy")

            for j in range(COLS):
                dsq = work.tile([128, P], f32, tag="dsq")
                nc.scalar.activation(dsq[:], bct_sb[:], Act.Square,
                                     bias=tneg[:, j:j + 1], scale=1.0)
                nc.vector.tensor_reduce(dy[:, j:j + 1], dsq[:],
                                        axis=mybir.AxisListType.X, op=Alu.min)
                nc.vector.tensor_tensor(acc_x[:], acc_x[:], dsq[:],
                                        op=Alu.min)

            # cham_y partials: masked sum of dy + valid counts
            dym = work.tile([128, COLS], f32, tag="dym")
            nc.vector.tensor_mul(dym[:], dy[:], valid[:])
            osum = consts.tile([128, 2], f32, tag="osum")
            nc.vector.tensor_reduce(osum[:, 0:1], dym[:],
                                    axis=mybir.AxisListType.X, op=Alu.add)
            nc.vector.tensor_reduce(osum[:, 1:2], valid[:],
                                    axis=mybir.AxisListType.X, op=Alu.add)

            nc.sync.dma_start(outx, acc_x[:])
            nc.sync.dma_start(outy, osum[:])

    nc.compile()
    return nc


def _get_program():
    if "nc" not in _CACHE:
        _CACHE["nc"] = _build_program()
    return _CACHE["nc"]


def kernel(bins, target_depth_maps):
    from concourse.bass_utils import run_bass_kernel_spmd

    bins = np.asarray(bins, dtype=np.float32)
    tdm = np.asarray(target_depth_maps, dtype=np.float32)

    bc = 0.5 * (bins[:, 1:] + bins[:, :-1])  # [4, 256]
    bct = np.ascontiguousarray(bc[np.arange(128) // PARTS_PER_BATCH])  # [128,256]
    tp = tdm.reshape(N, L)

    nc = _get_program()
    in_maps = []
    for c in range(N_CORES):
        shard = np.ascontiguousarray(
            tp[:, c * L_LOC:(c + 1) * L_LOC]).reshape(-1)
        in_maps.append({"tpd": shard, "bct": bct})

    res = run_bass_kernel_spmd(nc, in_maps, core_ids=list(range(N_CORES)))
    outs = res.results

    accx = np.stack([o["outx"] for o in outs])  # [8, 128, 256]
    osum = np.stack([o["outy"] for o in outs])  # [8, 128, 2]

    total = np.float64(0.0)
    for n in range(N):
        sl = slice(n * PARTS_PER_BATCH, (n + 1) * PARTS_PER_BATCH)
        cham_x = accx[:, sl, :].min(axis=(0, 1)).mean()
        dsum = osum[:, sl, 0].sum()
        cnt = osum[:, sl, 1].sum()
        cham_y = dsum / cnt
        total += cham_x + cham_y
    return np.float32(total / N)
